# revision 9
# baseline (speedup 1.0000x reference)
"""Trainium2 Bass kernel for a 2-layer GCN encoder with global mean pool.

Sharding: dst-partition of nodes across 8 NeuronCores (12500 nodes/core,
padded to 12544 slots = 49 blocks of 256). Both convs share ONE edge layout:
x is permuted host-side into the same table-row order that conv2's h1 table
uses (row = owner*12544 + block*256 + slot), so the chunk of an edge
(= src_core//2) and therefore the packed stream, gather indices and one-hot
metadata are identical for conv1 and conv2 and are uploaded once.
Self-loops are plain edges in the stream (src == dst), so the whole conv is
gather + one-hot matmuls with no separate self-loop path.

Each conv gathers source rows from a replicated bf16 DRAM table via
dma_gather (52 large gathers per conv, issued one group ahead so SWDGE prep
overlaps the previous group's compute), scatters each 128-edge tile into a
[128,512] PSUM super-block with TensorE matmuls against a bf16 one-hot
("valhot" = (iota==dstslot) * rsqrt(deg_src)) built by one fused
tensor_scalar (4x DVE mode; ~1/7 offloaded to GpSimd). The 1/sqrt(deg_dst)
scale, bias and ReLU are applied after a bf16 128x128 GEMM, with ReLU +
bf16 cast on the otherwise idle Scalar engine. h1 is AllGather-ed in bf16
between the convs; per-graph sums ride a batch-id one-hot into PSUM and are
combined with a small bf16 AllReduce before the two linear heads.

All floating-point math runs on device; the host only prepares integer
index/degree metadata, permuted/bf16-cast copies of inputs, and the packing.
"""
import sys

sys.path.insert(0, "/opt/trn_rl_repo")

import numpy as np
import ml_dtypes

N = 100000
E = 1600000
G = 256
NCORES = 8
NSHARD = N // NCORES            # 12500 real nodes per core
NPAD = 12544                    # padded shard size (= 49*256 = 98*128)
BLK = 256                       # block width (valhot/psum column range)
NBLK = NPAD // BLK              # 49 blocks per core
NSUB = NPAD // 128              # 98 GEMM sub-blocks per core
CH = 4                          # src chunks (int16 gather index limit)
W = 2 * NPAD                    # 25088 table rows per chunk window
TCELL = 9                       # tiles per (block, chunk) cell
CSLOT = TCELL * 128             # 1152 edge slots per cell
NTILES = NBLK * CH * TCELL      # 1764 tiles per conv per core
NSLOT = NTILES * 128            # 225792 edge slots per conv per core
SBS = [(s * 2, 2) for s in range(24)] + [(48, 1)]   # super-blocks of blocks
# gather groups: consecutive super-blocks, 2 SBs (4 blocks) per group
GGS = [SBS[i:i + 2] for i in range(0, len(SBS), 2)]  # 12x2 + 1x1
F = 128
FO = 64

# stream offset of cell (block b, chunk k): layout [group][chunk][block]
CELL_OFF = np.zeros((NBLK, CH), np.int64)
GOFF = []        # per group: (stream offset per chunk, first block, nblocks)
_base = 0
for _g in GGS:
    _blocks = [b for (b0, nb) in _g for b in range(b0, b0 + nb)]
    _gofs = []
    for _k in range(CH):
        _gofs.append(_base)
        for _b in _blocks:
            CELL_OFF[_b, _k] = _base
            _base += CSLOT
    GOFF.append((_gofs, _blocks[0], len(_blocks)))
assert _base == NSLOT

_CACHE = {}


def _pack_core(deg_tot, cnt4, seed=0):
    """Assign the core's NSHARD dsts to NBLK blocks of <=BLK slots so that no
    (block, chunk) cell exceeds CSLOT edges. LPT greedy (largest total degree
    first, block = argmin of projected max cell), then swap-repair."""
    rng = np.random.default_rng(seed)
    order = np.argsort(-deg_tot, kind="stable")
    block_of = np.empty(NSHARD, np.int64)
    loads = np.zeros((NBLK, CH), np.int64)
    counts = np.zeros(NBLK, np.int64)
    for n in order:
        c = cnt4[n]
        key = (loads + c).max(axis=1) * 100000 + loads.sum(axis=1)
        key[counts >= BLK] = 1 << 62
        b = int(np.argmin(key))
        block_of[n] = b
        loads[b] += c
        counts[b] += 1
    for _ in range(8000):
        mx = loads.max()
        if mx <= CSLOT:
            return block_of
        b, j = np.unravel_index(np.argmax(loads), loads.shape)
        members = np.where(block_of == b)[0]
        msort = members[np.argsort(-cnt4[members, j])]
        moved = False
        for n in msort[:10]:
            vn = cnt4[n]
            best = None
            for b2 in range(NBLK):
                if b2 == b:
                    continue
                mem2 = np.where(block_of == b2)[0]
                v2 = cnt4[mem2]
                nb = loads[b] - vn[None, :] + v2
                nb2 = loads[b2] + vn[None, :] - v2
                s = np.maximum(nb.max(axis=1), nb2.max(axis=1))
                k = int(np.argmin(s))
                if best is None or s[k] < best[0]:
                    best = (s[k], mem2[k], b2)
            if best is not None and best[0] < mx:
                _, n2, b2 = best
                block_of[n], block_of[n2] = b2, b
                loads[b] += cnt4[n2] - vn
                loads[b2] += vn - cnt4[n2]
                moved = True
                break
        if not moved:
            n = rng.choice(members)
            b2 = int(rng.integers(NBLK))
            if b2 == b:
                continue
            mem2 = np.where(block_of == b2)[0]
            n2 = rng.choice(mem2)
            block_of[n], block_of[n2] = b2, b
            loads[b] += cnt4[n2] - cnt4[n]
            loads[b2] += cnt4[n] - cnt4[n2]
    raise RuntimeError("cell packing failed; raise TCELL")


def _host_prep(x, edge_index, batch):
    srcF = edge_index[0].astype(np.int64)
    dstF = edge_index[1].astype(np.int64)
    # degrees include the self-loop (+1); self-loop messages are injected
    # on-device from the local table shard, not via the gather stream
    deg = np.bincount(dstF, minlength=N).astype(np.int64) + 1

    owner_e = dstF // NSHARD
    chunk_e = srcF // NSHARD // 2         # = tablerow(src) // W, packing-free

    # --- pack every core's dsts into blocks ---------------------------------
    block_of_g = np.empty(N, np.int64)
    slot_of_g = np.empty(N, np.int64)
    for c in range(NCORES):
        base = c * NSHARD
        m = owner_e == c
        ed = dstF[m] - base
        cnt4 = np.bincount(
            ed * CH + chunk_e[m], minlength=NSHARD * CH
        ).reshape(NSHARD, CH)
        blk = _pack_core(deg[base : base + NSHARD], cnt4)
        block_of_g[base : base + NSHARD] = blk
        # slot within block: stable order of nodes per block
        o = np.argsort(blk, kind="stable")
        r = np.empty(NSHARD, np.int64)
        r[o] = np.arange(NSHARD) - np.searchsorted(blk[o], blk[o])
        slot_of_g[base : base + NSHARD] = r
        assert r.max() < BLK

    node_owner = np.arange(N) // NSHARD
    tablerow = node_owner * NPAD + block_of_g * BLK + slot_of_g  # per node

    degf = deg.astype(np.float32)
    dstslot = tablerow % BLK              # position of a dst inside its block

    # permuted bf16 x table, shared by all cores
    x_tab = np.zeros((NPAD * NCORES, F), ml_dtypes.bfloat16)
    x_tab[tablerow] = x.astype(ml_dtypes.bfloat16)

    per_core = []
    for c in range(NCORES):
        base = c * NSHARD
        m = owner_e == c
        es, ed = srcF[m], dstF[m]
        eblk = block_of_g[ed]
        idxval = tablerow[es] % W

        cell = eblk * CH + chunk_e[m]
        o = np.argsort(cell, kind="stable")
        cell_s = cell[o]
        cnt = np.bincount(cell_s, minlength=NBLK * CH)
        if cnt.max() > CSLOT:
            raise RuntimeError("cell overflow; raise TCELL")
        starts = np.zeros(NBLK * CH, np.int64)
        starts[1:] = np.cumsum(cnt)[:-1]
        rank = np.arange(len(cell_s)) - starts[cell_s]
        pos = CELL_OFF.reshape(-1)[cell_s] + rank

        idxv = np.zeros(NSLOT, np.int16)
        dlv = np.full(NSLOT, -1.0, np.float32)
        dgv = np.ones(NSLOT, np.float32)
        idxv[pos] = idxval[o].astype(np.int16)
        dlv[pos] = dstslot[ed[o]].astype(np.float32)
        dgv[pos] = degf[es[o]]

        core = {}
        wrapped = np.ascontiguousarray(idxv.reshape(-1, 16).T)  # [16, NSLOT/16]
        core["idx"] = np.tile(wrapped, (8, 1))                  # [128, NSLOT/16]
        core["dl"] = np.ascontiguousarray(dlv.reshape(-1, 128).T)  # [128,NTILES]
        core["dg"] = np.ascontiguousarray(dgv.reshape(-1, 128).T)

        # per-slot node metadata in [slot%128, slot//128] layout
        nodes = np.arange(base, base + NSHARD)
        slotidx = block_of_g[nodes] * BLK + slot_of_g[nodes]
        degd = np.ones(NPAD, np.float32)
        degd[slotidx] = degf[nodes]
        blv = np.full(NPAD, -1.0, np.float32)
        blv[slotidx] = batch[nodes].astype(np.float32)
        core["degd"] = np.ascontiguousarray(degd.reshape(NSUB, 128).T)
        core["bl"] = np.ascontiguousarray(blv.reshape(NSUB, 128).T)
        core["x_perm"] = x_tab[c * NPAD : (c + 1) * NPAD]
        per_core.append(core)

    return per_core, x_tab


def _build_bass():
    from concourse import bacc, tile, bass
    import concourse.mybir as mybir

    F32 = mybir.dt.float32
    BF16 = mybir.dt.bfloat16
    I16 = mybir.dt.int16
    EQ = mybir.AluOpType.is_equal
    MULT = mybir.AluOpType.mult
    ADD = mybir.AluOpType.add
    MAX = mybir.AluOpType.max
    AF = mybir.ActivationFunctionType

    nc = bacc.Bacc("TRN2", target_bir_lowering=False, debug=False,
                   num_devices=NCORES)

    x_tab = nc.dram_tensor("x_tab", [NPAD * NCORES, F], BF16,
                           kind="ExternalInput")
    x_perm_d = nc.dram_tensor("x_perm", [NPAD, F], BF16, kind="ExternalInput")
    pcol_d = nc.dram_tensor("pcol", [128, 1], F32, kind="ExternalInput")
    idx_d = nc.dram_tensor("idx", [128, NSLOT // 16], I16,
                           kind="ExternalInput")
    dl_d = nc.dram_tensor("dl", [128, NTILES], F32, kind="ExternalInput")
    dg_d = nc.dram_tensor("dg", [128, NTILES], F32, kind="ExternalInput")
    iota_d = nc.dram_tensor("iota", [128, 256], BF16, kind="ExternalInput")
    degd_d = nc.dram_tensor("degd", [128, NSUB], F32, kind="ExternalInput")
    bl_d = nc.dram_tensor("bl", [128, NSUB], F32, kind="ExternalInput")
    w_d = [nc.dram_tensor(f"w{i+1}", [F, F], BF16, kind="ExternalInput")
           for i in range(2)]
    bbc_d = [nc.dram_tensor(f"b{i+1}bc", [128, F], F32, kind="ExternalInput")
             for i in range(2)]
    wmu_d = nc.dram_tensor("wmu", [F, FO], BF16, kind="ExternalInput")
    wlv_d = nc.dram_tensor("wlv", [F, FO], BF16, kind="ExternalInput")
    bmu_d = nc.dram_tensor("bmubc", [128, FO], F32, kind="ExternalInput")
    blv_d = nc.dram_tensor("blvbc", [128, FO], F32, kind="ExternalInput")
    cnt_d = nc.dram_tensor("cnt", [128, 2], F32, kind="ExternalInput")

    mu_o = nc.dram_tensor("mu", [G, FO], F32, kind="ExternalOutput")
    lv_o = nc.dram_tensor("lv", [G, FO], F32, kind="ExternalOutput")

    with tile.TileContext(nc) as tc:
        with (
            tc.tile_pool(name="const", bufs=1) as cp,
            tc.tile_pool(name="stream", bufs=8) as sp,
            tc.tile_pool(name="selfp", bufs=4) as xp,
            tc.tile_pool(name="work", bufs=6) as wp,
            tc.tile_pool(name="vhp", bufs=8) as vp,
            tc.tile_pool(name="psum", bufs=2, space="PSUM") as pp,
            tc.tile_pool(name="psum3", bufs=3, space="PSUM") as pp3,
            tc.tile_pool(name="psum1", bufs=1, space="PSUM") as pp1,
            tc.tile_pool(name="dram", bufs=1, space="DRAM") as dp,
        ):
            # ---- constants; ordered so the gather/vh path unblocks first ---
            iota = cp.tile([128, 256], BF16, tag="iota")
            nc.sync.dma_start(iota[:], iota_d[:])
            pcol = cp.tile([128, 1], F32, tag="pcol")
            nc.sync.dma_start(pcol[:], pcol_d[:])
            idxfull = cp.tile([128, NSLOT // 16], I16, tag="idxfull")
            nc.sync.dma_start(idxfull[:], idx_d[:])
            dl_sb = cp.tile([128, NTILES], F32, tag="dl")
            nc.sync.dma_start(dl_sb[:], dl_d[:])
            # per-edge v = 1/sqrt(max(deg_src,1)), shared by both convs
            dg = cp.tile([128, NTILES], F32, tag="dg")
            nc.sync.dma_start(dg[:], dg_d[:])
            v_sb = cp.tile([128, NTILES], F32, tag="v")
            nc.vector.tensor_scalar(dg[:], dg[:], 1.0, None, MAX)
            nc.scalar.activation(dg[:], dg[:], AF.Sqrt)
            nc.vector.reciprocal(v_sb[:], dg[:])

            zeros = cp.tile([128, 512], BF16, tag="zeros")
            nc.vector.memset(zeros[:], 0.0)
            w_sb = [cp.tile([F, F], BF16, tag=f"w{i}", name=f"w{i}")
                    for i in range(2)]
            bbc_sb = [cp.tile([128, F], F32, tag=f"bbc{i}", name=f"bbc{i}")
                      for i in range(2)]
            for i in range(2):
                nc.sync.dma_start(w_sb[i][:], w_d[i][:])
                nc.sync.dma_start(bbc_sb[i][:], bbc_d[i][:])

            # dinv over the dst shard: 1/sqrt(max(deg,1))
            degd = cp.tile([128, NSUB], F32, tag="degd")
            nc.sync.dma_start(degd[:], degd_d[:])
            dinvd = cp.tile([128, NSUB], F32, tag="dinvd")
            nc.vector.tensor_scalar(degd[:], degd[:], 1.0, None, MAX)
            nc.scalar.activation(degd[:], degd[:], AF.Sqrt)
            nc.vector.reciprocal(dinvd[:], degd[:])

            bl_sb = cp.tile([128, NSUB], F32, tag="bl")
            nc.sync.dma_start(bl_sb[:], bl_d[:])

            wmu = cp.tile([F, FO], BF16, tag="wmu")
            wlv = cp.tile([F, FO], BF16, tag="wlv")
            bmu = cp.tile([128, FO], F32, tag="bmu")
            blv = cp.tile([128, FO], F32, tag="blv")
            for t, d in [(wmu, wmu_d), (wlv, wlv_d), (bmu, bmu_d), (blv, blv_d)]:
                nc.sync.dma_start(t[:], d[:])

            # cnt -> 1/max(cnt,1)
            cnt = cp.tile([128, 2], F32, tag="cnt")
            nc.sync.dma_start(cnt[:], cnt_d[:])
            rcnt = cp.tile([128, 2], F32, tag="rcnt")
            nc.vector.tensor_scalar(cnt[:], cnt[:], 1.0, None, MAX)
            nc.vector.reciprocal(rcnt[:], cnt[:])

            # ---- DRAM intermediates ---------------------------------------
            h1_shard = dp.tile([NPAD, F], BF16)
            h1_full = dp.tile([NPAD * NCORES, F], BF16)
            sums_in = dp.tile([128, 256], BF16)
            sums_out = dp.tile([128, 256], BF16)

            pool_ps = pp1.tile([128, 256], F32, tag="pool", name="pool_ps")
            vh_count = [0]

            def issue_gathers(table, gofs, nbg):
                msgs = []
                for k in range(CH):
                    clen = nbg * CSLOT
                    msg = sp.tile([128, nbg * TCELL, F], BF16, tag="msg")
                    nc.gpsimd.dma_gather(
                        msg[:, : nbg * TCELL, :],
                        table[W * k :, :],
                        idxfull[:, gofs[k] // 16 : (gofs[k] + clen) // 16],
                        clen, clen, F, elem_step=F,
                        single_packet=False,
                    )
                    msgs.append(msg.rearrange("p t f -> p (t f)"))
                return msgs

            def process_group(conv, msgs, b0g, nbg, selftab, writer):
                first_sb = next(i for i, (b0, nb) in enumerate(SBS)
                                if b0 == b0g)
                n_sbs = 1 if nbg == 1 else 2
                for si in range(first_sb, first_sb + n_sbs):
                    b0, nb = SBS[si]
                    agg = pp3.tile([128, 512], F32, tag="agg")
                    # HW: start=True clears has_written for the WHOLE psum
                    # bank — one full-width start matmul per bank.
                    nc.tensor.matmul(agg[:], zeros[:, :128], zeros[:],
                                     start=True, stop=False)
                    # self-loop term per 128-sub-block:
                    # agg[:, sub] = selftab_block^T @ diag(dinv)
                    for sub in range(nb * 2):
                        b128 = b0 * 2 + sub
                        xl = xp.tile([128, F], BF16, tag="xl")
                        nc.scalar.dma_start(
                            xl[:], selftab[b128 * 128 : (b128 + 1) * 128, :]
                        )
                        diag = wp.tile([128, 128], BF16, tag="diag")
                        nc.vector.tensor_scalar(
                            diag[:], iota[:, :128], pcol[:],
                            dinvd[:, b128 : b128 + 1], EQ, MULT,
                        )
                        nc.tensor.matmul(
                            agg[:, sub * 128 : (sub + 1) * 128],
                            xl[:], diag[:], start=False, stop=False,
                        )
                    for k in range(CH):
                        m2 = msgs[k]
                        for bi in range(nb):
                            b = b0 + bi
                            bofs = b - b0g   # block index within group
                            for t in range(TCELL):
                                tl = bofs * TCELL + t
                                col = CELL_OFF[b, k] // 128 + t
                                vh = vp.tile([128, 256], BF16, tag="vh")
                                eng = (nc.gpsimd if vh_count[0] % 7 == 6
                                       else nc.vector)
                                vh_count[0] += 1
                                eng.tensor_scalar(
                                    vh[:], iota[:],
                                    dl_sb[:, col : col + 1],
                                    v_sb[:, col : col + 1], EQ, MULT,
                                )
                                nc.tensor.matmul(
                                    agg[:, bi * 256 : (bi + 1) * 256],
                                    m2[:, tl * 128 : (tl + 1) * 128],
                                    vh[:],
                                    start=False,
                                    stop=(k == CH - 1 and bi == nb - 1
                                          and t == TCELL - 1),
                                )
                    aggT = wp.tile([128, 512], BF16, tag="aggT")
                    nc.scalar.activation(
                        aggT[:, : nb * 256], agg[:, : nb * 256], AF.Copy
                    )
                    for sub in range(nb * 2):
                        b128 = b0 * 2 + sub
                        gm = pp.tile([128, F], F32, tag="gemm")
                        nc.tensor.matmul(
                            gm[:], aggT[:, sub * 128 : (sub + 1) * 128],
                            w_sb[conv][:], start=True, stop=True,
                        )
                        writer(b128, gm)

            def run_conv(conv, table, selftab, writer):
                pend = None
                for gofs, b0g, nbg in GOFF:
                    msgs = issue_gathers(table, gofs, nbg)
                    if pend is not None:
                        process_group(conv, *pend, selftab, writer)
                    pend = (msgs, b0g, nbg)
                process_group(conv, *pend, selftab, writer)

            def w_conv1(b, gm):
                h = wp.tile([128, F], F32, tag="h")
                nc.vector.scalar_tensor_tensor(
                    h[:], gm[:], dinvd[:, b : b + 1], bbc_sb[0][:], MULT, ADD,
                )
                hb = wp.tile([128, F], BF16, tag="hb")
                nc.scalar.activation(hb[:], h[:], AF.Relu)
                nc.sync.dma_start(h1_shard[b * 128 : (b + 1) * 128, :], hb[:])

            def w_conv2(b, gm):
                h = wp.tile([128, F], F32, tag="h")
                nc.vector.scalar_tensor_tensor(
                    h[:], gm[:], dinvd[:, b : b + 1], bbc_sb[1][:], MULT, ADD,
                )
                hb = wp.tile([128, F], BF16, tag="hb")
                nc.scalar.activation(hb[:], h[:], AF.Relu)
                ph = vp.tile([128, 256], BF16, tag="ph")
                nc.vector.tensor_scalar(
                    ph[:], iota[:], bl_sb[:, b : b + 1], None, EQ,
                )
                nc.tensor.matmul(
                    pool_ps[:], hb[:], ph[:],
                    start=(b == 0), stop=(b == NSUB - 1),
                )

            run_conv(0, x_tab, x_perm_d, w_conv1)

            # conv1 writes only a per-core shard; gather it for conv2's table
            nc.gpsimd.collective_compute(
                "AllGather", mybir.AluOpType.bypass,
                replica_groups=[list(range(NCORES))],
                ins=[h1_shard.opt()], outs=[h1_full.opt()],
            )
            run_conv(1, h1_full, h1_shard, w_conv2)

            # ---- pooling sums AllReduce + heads ---------------------------
            pool_sb = wp.tile([128, 256], BF16, tag="poolsb")
            nc.vector.tensor_copy(pool_sb[:], pool_ps[:])
            nc.sync.dma_start(sums_in[:], pool_sb[:])
            nc.gpsimd.collective_compute(
                "AllReduce", mybir.AluOpType.add,
                replica_groups=[list(range(NCORES))],
                ins=[sums_in.opt()], outs=[sums_out.opt()],
            )
            sums_sb = wp.tile([128, 256], BF16, tag="sums")
            nc.sync.dma_start(sums_sb[:], sums_out[:])
            for j in range(2):
                for wt, bt, out_d in [(wmu, bmu, mu_o), (wlv, blv, lv_o)]:
                    hp = pp.tile([128, FO], F32, tag="head")
                    nc.tensor.matmul(
                        hp[:], sums_sb[:, j * 128 : (j + 1) * 128], wt[:],
                        start=True, stop=True,
                    )
                    hs = wp.tile([128, FO], F32, tag="headsb")
                    nc.vector.scalar_tensor_tensor(
                        hs[:], hp[:], rcnt[:, j : j + 1], bt[:], MULT, ADD,
                    )
                    nc.sync.dma_start(
                        out_d[j * 128 : (j + 1) * 128, :], hs[:])

    nc.compile()
    return nc


def kernel(x, edge_index, batch, W1, b1, W2, b2, W_mu, b_mu, W_lv, b_lv):
    from concourse import bass_utils

    x = np.asarray(x, dtype=np.float32)
    edge_index = np.asarray(edge_index)
    batch = np.asarray(batch)

    per_core, x_tab = _host_prep(x, edge_index, batch)

    iota = np.broadcast_to(
        np.arange(256, dtype=np.float32), (128, 256)
    ).astype(ml_dtypes.bfloat16).copy()
    cnts = np.bincount(np.asarray(batch, np.int64), minlength=G).astype(np.float32)
    cnt_arr = np.ascontiguousarray(cnts.reshape(2, 128).T)
    shared = dict(
        x_tab=x_tab,
        iota=iota,
        pcol=np.arange(128, dtype=np.float32).reshape(128, 1),
        w1=np.asarray(W1, np.float32).astype(ml_dtypes.bfloat16),
        w2=np.asarray(W2, np.float32).astype(ml_dtypes.bfloat16),
        b1bc=np.broadcast_to(np.asarray(b1, np.float32), (128, F)).copy(),
        b2bc=np.broadcast_to(np.asarray(b2, np.float32), (128, F)).copy(),
        wmu=np.asarray(W_mu, np.float32).astype(ml_dtypes.bfloat16),
        wlv=np.asarray(W_lv, np.float32).astype(ml_dtypes.bfloat16),
        bmubc=np.broadcast_to(np.asarray(b_mu, np.float32), (128, FO)).copy(),
        blvbc=np.broadcast_to(np.asarray(b_lv, np.float32), (128, FO)).copy(),
        cnt=cnt_arr,
    )
    in_maps = [dict(shared, **pc) for pc in per_core]

    if "nc" not in _CACHE:
        _CACHE["nc"] = _build_bass()
    nc = _CACHE["nc"]

    import os as _os
    res = bass_utils.run_bass_kernel_spmd(
        nc, in_maps, core_ids=list(range(NCORES)),
        trace=_os.environ.get("KTRACE") == "1",
    )
    _CACHE["last_res"] = res
    r0 = res.results[0]
    return (r0["mu"].copy(), r0["lv"].copy())
